# revision 6
# baseline (speedup 1.0000x reference)
"""Multi-head attention (GQA, 32 q-heads / 8 kv-heads, S=2048, H=4096) on 8
Trainium2 NeuronCores.

Sharding: tensor-parallel across heads. Core c owns kv-head c and q-heads
4c..4c+3 (Wq/Wk/Wv column-sharded, Wo row-sharded). Each core computes a
partial output [S, H]; the host sums the 8 partials.

Per-core dataflow (everything bf16 into the PE, fp32 accumulation):
  A) qT/kT/vT = W.T @ hiddenT  (weights stationary, hiddenT moving)
     + RoPE applied in the transposed [hd, s] layout
     + vT transposed back to natural v[s, hd] via PE-transpose
  B) per q-head: scoresT[j,i] = kT.T @ qT  ->  E = exp(scale*scoresT)
     denom[i] = onesT @ E (PE),  attnT[d,i] = v.T @ E, normalized on DVE
  C) partial_out[s,:] = attnT.T @ Wo_c  (attnT stationary, Wo moving)
"""

import math
import os
import sys

if os.path.isdir("/opt/trn_rl_repo") and "/opt/trn_rl_repo" not in sys.path:
    sys.path.insert(0, "/opt/trn_rl_repo")

import numpy as np
import ml_dtypes

import concourse.bacc as bacc
import concourse.mybir as mybir
from concourse import tile
from concourse.bass_utils import run_bass_kernel_spmd

BF16 = mybir.dt.bfloat16
F32 = mybir.dt.float32
NPBF16 = ml_dtypes.bfloat16

S = 2048
H = 4096
HD = 128
NH = 32
NKV = 8
N_CORES = 8
QH = NH // N_CORES          # q-heads per core = 4
F = QH * HD                 # q feature columns per core = 512
KT = H // 128               # contraction tiles for the projections = 32
ST = S // 128               # 128-row tiles along S = 16
SG = S // 512               # 512-wide groups along S = 4
SCALE = 1.0 / math.sqrt(HD)

_BUILT = {}


def _build(masked: bool):
    nc = bacc.Bacc(None, target_bir_lowering=False)

    hT = nc.declare_dram_parameter("hT", [H, S], BF16, isOutput=False)
    wqkv = nc.declare_dram_parameter("wqkv", [H, F + 2 * HD], BF16, isOutput=False)
    wo = nc.declare_dram_parameter("wo", [F, H], BF16, isOutput=False)
    cosT = nc.declare_dram_parameter("cosT", [HD, S], F32, isOutput=False)
    sinTe = nc.declare_dram_parameter("sinTe", [HD, S], F32, isOutput=False)
    eye = nc.declare_dram_parameter("eye", [128, 128], BF16, isOutput=False)
    if masked:
        maskT = nc.declare_dram_parameter("maskT", [S, S], F32, isOutput=False)
    out = nc.declare_dram_parameter("out", [S, H], F32, isOutput=True)

    FW = F + 2 * HD  # 768 weight columns per contraction tile

    with tile.TileContext(nc) as tc:
        with tc.tile_pool(name="persist", bufs=1) as pp:
            # persistent SBUF tensors
            cos_sb = pp.tile([HD, S], F32, tag="cos")
            sin_sb = pp.tile([HD, S], F32, tag="sin")
            eye_sb = pp.tile([128, 128], BF16, tag="eye")
            ones_sb = pp.tile([128, 128], BF16, tag="ones")
            qT_sb = [pp.tile([HD, S], BF16, tag=f"qT{h}", name=f"qT{h}") for h in range(QH)]
            kT_sb = pp.tile([HD, S], BF16, tag="kT")
            v_sb = pp.tile([128, ST * HD], BF16, tag="v")  # block jt: v[jt*128:(jt+1)*128, :]
            aT_sb = [pp.tile([HD, S], BF16, tag=f"aT{h}", name=f"aT{h}") for h in range(QH)]

            nc.sync.dma_start(cos_sb[:], cosT[:])
            nc.sync.dma_start(sin_sb[:], sinTe[:])
            nc.sync.dma_start(eye_sb[:], eye[:])
            nc.gpsimd.memset(ones_sb[:], 1.0)

            # ---------------- Phase A: projections + RoPE ----------------
            with (
                tc.tile_pool(name="phA_sb", bufs=1) as pa,
                tc.tile_pool(name="phA_h", bufs=2) as pah,
                tc.tile_pool(name="phA_tmp", bufs=3) as pat,
                tc.tile_pool(name="phA_ps", bufs=3, space="PSUM") as pap,
                tc.tile_pool(name="phA_pst", bufs=2, space="PSUM") as papt,
            ):
                w_sb = pa.tile([128, KT * FW], BF16, tag="wqkv")
                nc.sync.dma_start(
                    w_sb[:].rearrange("p (a f) -> p a f", a=KT),
                    wqkv[:].rearrange("(a p) f -> p a f", p=128),
                )
                for sg in range(SG):
                    hc = pah.tile([128, KT * 512], BF16, tag="hc")
                    nc.sync.dma_start(
                        hc[:].rearrange("p (a s) -> p a s", a=KT),
                        hT[:, sg * 512:(sg + 1) * 512].rearrange(
                            "(a p) s -> p a s", p=128
                        ),
                    )
                    for f in range(QH + 2):
                        ps = pap.tile([128, 512], F32, tag="proj")
                        for k in range(KT):
                            nc.tensor.matmul(
                                ps[:],
                                w_sb[:, k * FW + f * 128:k * FW + (f + 1) * 128],
                                hc[:, k * 512:(k + 1) * 512],
                                start=(k == 0),
                                stop=(k == KT - 1),
                            )
                        sl = slice(sg * 512, (sg + 1) * 512)
                        if f < QH + 1:
                            # RoPE: out[d] = x[d]*cos[d] + x[(d+64)%128]*sinTe[d]
                            dest = (qT_sb[f] if f < QH else kT_sb)[:, sl]
                            t1 = pat.tile([128, 512], F32, tag="t1")
                            t2 = pat.tile([128, 512], F32, tag="t2")
                            nc.vector.tensor_mul(t1[:], ps[:], cos_sb[:, sl])
                            nc.vector.tensor_mul(
                                t2[0:64, :], ps[64:128, :], sin_sb[0:64, sl]
                            )
                            nc.vector.tensor_mul(
                                t2[64:128, :], ps[0:64, :], sin_sb[64:128, sl]
                            )
                            nc.vector.tensor_add(dest, t1[:], t2[:])
                        else:
                            # v: evict then PE-transpose into natural layout
                            vt = pat.tile([128, 512], BF16, tag="vt")
                            nc.vector.tensor_copy(vt[:], ps[:])
                            for b in range(4):
                                jt = sg * 4 + b
                                pst = papt.tile([128, 128], BF16, tag="vtr")
                                nc.tensor.transpose(
                                    pst[:], vt[:, b * 128:(b + 1) * 128], eye_sb[:]
                                )
                                nc.vector.tensor_copy(
                                    v_sb[:, jt * HD:(jt + 1) * HD], pst[:]
                                )

            # ---------------- Phase B: attention per head ----------------
            with (
                tc.tile_pool(name="phB_E", bufs=20) as pe_pool,
                tc.tile_pool(name="phB_tmp", bufs=3) as pbt,
                tc.tile_pool(name="phB_m", bufs=3) as pbm,
                tc.tile_pool(name="phB_s", bufs=3, space="PSUM") as pbs,
                tc.tile_pool(name="phB_acc", bufs=2, space="PSUM") as pba,
            ):
                for h in range(QH):
                    qh = qT_sb[h]
                    et = [pe_pool.tile([128, S], BF16, tag="E", name=f"E{h}_{j}") for j in range(ST)]
                    for ig in range(SG):
                        isl = slice(ig * 512, (ig + 1) * 512)
                        for jt in range(ST):
                            sps = pbs.tile([128, 512], F32, tag="s")
                            nc.tensor.matmul(
                                sps[:],
                                kT_sb[:, jt * 128:(jt + 1) * 128],
                                qh[:, isl],
                                start=True,
                                stop=True,
                            )
                            if masked:
                                # host pre-scales maskT by sqrt(HD):
                                # exp(SCALE*(scores + maskT)) == softmax logits
                                mt = pbm.tile([128, 512], F32, tag="mT")
                                nc.sync.dma_start(
                                    mt[:], maskT[jt * 128:(jt + 1) * 128, isl]
                                )
                                sm = pbm.tile([128, 512], F32, tag="sm")
                                nc.vector.tensor_add(sm[:], sps[:], mt[:])
                                nc.scalar.activation(
                                    et[jt][:, isl], sm[:],
                                    mybir.ActivationFunctionType.Exp,
                                    scale=SCALE,
                                )
                            else:
                                nc.scalar.activation(
                                    et[jt][:, isl], sps[:],
                                    mybir.ActivationFunctionType.Exp,
                                    scale=SCALE,
                                )
                        den = pba.tile([128, 512], F32, tag="den")
                        for jt in range(ST):
                            nc.tensor.matmul(
                                den[:], ones_sb[:], et[jt][:, isl],
                                start=(jt == 0), stop=(jt == ST - 1),
                            )
                        pv = pba.tile([128, 512], F32, tag="pv")
                        for jt in range(ST):
                            nc.tensor.matmul(
                                pv[:], v_sb[:, jt * HD:(jt + 1) * HD], et[jt][:, isl],
                                start=(jt == 0), stop=(jt == ST - 1),
                            )
                        rc = pbt.tile([128, 512], F32, tag="rc")
                        nc.vector.reciprocal(rc[:], den[:])
                        nc.vector.tensor_mul(aT_sb[h][:, isl], pv[:], rc[:])

            # ---------------- Phase C: output projection ----------------
            with (
                tc.tile_pool(name="phC_sb", bufs=1) as pc,
                tc.tile_pool(name="phC_o", bufs=4) as pco,
                tc.tile_pool(name="phC_ps", bufs=4, space="PSUM") as pcp,
            ):
                wo_sb = pc.tile([128, QH * H], BF16, tag="wo")
                nc.sync.dma_start(
                    wo_sb[:].rearrange("p (a o) -> p a o", a=QH),
                    wo[:].rearrange("(a p) o -> p a o", p=128),
                )
                for st in range(ST):
                    ssl = slice(st * 128, (st + 1) * 128)
                    for ho in range(H // 512):
                        po = pcp.tile([128, 512], F32, tag="o")
                        for f4 in range(QH):
                            nc.tensor.matmul(
                                po[:],
                                aT_sb[f4][:, ssl],
                                wo_sb[:, f4 * H + ho * 512:f4 * H + (ho + 1) * 512],
                                start=(f4 == 0),
                                stop=(f4 == QH - 1),
                            )
                        ob = pco.tile([128, 512], F32, tag="ob")
                        nc.scalar.copy(ob[:], po[:])
                        nc.sync.dma_start(
                            out[ssl, ho * 512:(ho + 1) * 512], ob[:]
                        )

    nc.finalize()
    return nc


def _get_kernel(masked: bool):
    if masked not in _BUILT:
        _BUILT[masked] = _build(masked)
    return _BUILT[masked]


def kernel(hidden_states, position_ids, attention_mask, cos, sin, Wq, Wk, Wv, Wo,
           _collect_exec_info=None):
    hidden_states = np.asarray(hidden_states)
    attention_mask = np.asarray(attention_mask)
    cos = np.asarray(cos)
    sin = np.asarray(sin)
    Wq, Wk, Wv, Wo = (np.asarray(a) for a in (Wq, Wk, Wv, Wo))

    masked = bool(np.any(attention_mask))
    nc = _get_kernel(masked)

    hT = np.ascontiguousarray(hidden_states[0].T).astype(NPBF16)
    cosT = np.ascontiguousarray(cos[0].T).astype(np.float32)
    sinTe = np.ascontiguousarray(sin[0].T).astype(np.float32)
    sinTe[:64] = -sinTe[:64]
    eye = np.eye(128, dtype=NPBF16)

    in_maps = []
    for c in range(N_CORES):
        wqkv = np.concatenate(
            [
                Wq[:, c * F:(c + 1) * F],
                Wk[:, c * HD:(c + 1) * HD],
                Wv[:, c * HD:(c + 1) * HD],
            ],
            axis=1,
        ).astype(NPBF16)
        m = {
            "hT": hT,
            "wqkv": wqkv,
            "wo": Wo[c * F:(c + 1) * F, :].astype(NPBF16),
            "cosT": cosT,
            "sinTe": sinTe,
            "eye": eye,
        }
        if masked:
            m["maskT"] = (
                np.ascontiguousarray(attention_mask[0, 0].T).astype(np.float32)
                * math.sqrt(HD)
            )
        in_maps.append(m)

    trace = _collect_exec_info is not None
    res = run_bass_kernel_spmd(nc, in_maps, list(range(N_CORES)), trace=trace)
    if trace:
        _collect_exec_info["exec_time_ns"] = res.exec_time_ns
        _collect_exec_info["results"] = res

    acc = res.results[0]["out"].astype(np.float64)
    for c in range(1, N_CORES):
        acc += res.results[c]["out"].astype(np.float64)
    return acc.astype(np.float32)[None, :, :]
